# revision 1
# baseline (speedup 1.0000x reference)
"""BlockGRU Trainium2 kernel.

Block-diagonal GRU cell: 8 independent blocks (block_size 256), batch 2048,
input_dim 1024. Sharded one block per NeuronCore (8 cores).

Per-core layout: gates on partitions, batch on the free dimension
(everything transposed on the host, which is free). Matmul operands are
fp16 (measured end-to-end rel-L2 error vs the fp32 reference: 2.6e-4;
fp16 halves the DMA streams and runs the PE at full rate with fast
weight load); accumulation and all elementwise math stay fp32. r/z gate
pre-activations accumulate input-projection + hidden-projection directly
in PSUM; i_n and h_n are kept separate for the r-gating. Per-partition
biases fuse into ScalarE activation ops (sigmoid/tanh) and a
scalar_tensor_tensor on VectorE; 1-z runs on the idle GPSIMD engine.
"""

import sys

if "/opt/trn_rl_repo" not in sys.path:
    sys.path.insert(0, "/opt/trn_rl_repo")

import numpy as np

INPUT_DIM = 1024
HIDDEN_DIM = 2048
NUM_BLOCKS = 8
BS = HIDDEN_DIM // NUM_BLOCKS  # 256
G3 = 3 * BS                    # 768
BATCH = 2048
CHUNKS = [512, 512, 512, 256, 256]   # batch chunks (PSUM bank = 512 fp32;
                                     # small tail chunks shorten the post-PE tail)
KX = INPUT_DIM // 128          # 8 contraction tiles on the input side
KH = BS // 128                 # 2 contraction tiles on the hidden side
ST = BS // 128                 # 2 state partition-tiles per block

_cached = None


def _build():
    import concourse.tile as tile
    import concourse.mybir as mybir
    from concourse import bacc

    f32 = mybir.dt.float32
    f16 = mybir.dt.float16
    ALU = mybir.AluOpType
    ACT = mybir.ActivationFunctionType

    nc = bacc.Bacc("TRN2", target_bir_lowering=False, debug=False, num_devices=8)

    xT = nc.dram_tensor("xT", [INPUT_DIM, BATCH], f16, kind="ExternalInput")
    wih = nc.dram_tensor("wih", [INPUT_DIM, G3], f16, kind="ExternalInput")
    whh = nc.dram_tensor("whh", [BS, G3], f16, kind="ExternalInput")
    hT = nc.dram_tensor("hT", [BS, BATCH], f16, kind="ExternalInput")
    bias = nc.dram_tensor("bias", [128, 5 * ST], f32, kind="ExternalInput")
    oT = nc.dram_tensor("oT", [BS, BATCH], f32, kind="ExternalOutput")

    with tile.TileContext(nc) as tc:
        with (
            tc.tile_pool(name="const", bufs=1) as cp,
            tc.tile_pool(name="xin", bufs=3) as xp,
            tc.tile_pool(name="hin", bufs=3) as hp,
            tc.tile_pool(name="gates", bufs=4) as gp,
            tc.tile_pool(name="outs", bufs=3) as op,
            tc.tile_pool(name="psum", bufs=1, space="PSUM") as pp,
        ):
            # PE warm-up: harmless matmuls on a zeroed tile while the prefill
            # DMA runs, so the clock ramp (cold -> full rate) completes before
            # real work arrives. Uses the p0 PSUM slot ahead of chunk 0.
            wu = cp.tile([128, 32], f16, tag="wu")
            nc.vector.memset(wu[:], 0.0)
            pdummy = pp.tile([128, 32], f32, tag="p0", name="pdummy")
            for _ in range(48):
                nc.tensor.matmul(pdummy[0:32, :], wu[:, 0:32], wu[:],
                                 start=True, stop=True)

            # --- DMA prologue. The DMA queue is serial at HBM bandwidth, so
            # emission order == arrival order == PE consumption order: the
            # x-side weights and chunk-0 columns first (bulk of PE work),
            # hidden-side weights/state + biases after (consumed at the end
            # of chunk 0's accumulation). ---
            c0w = CHUNKS[0]
            cs0 = slice(0, c0w)
            wih_sb = []
            x0_t = []
            # k-tiles load pairwise-merged via 3D access patterns: fewer
            # DMA descriptors means the serial DMA stream outpaces PE's
            # k-major consumption, so chunk 0 runs stall-free.
            for kp in range(0, KX, 2):
                wm = cp.tile([128, 2 * G3], f16, tag=f"wih{kp}", name=f"wihm{kp}")
                nc.sync.dma_start(
                    wm[:].rearrange("p (k g) -> p k g", k=2),
                    wih.ap()[kp * 128:(kp + 2) * 128, :]
                        .rearrange("(k p) g -> p k g", p=128))
                wih_sb.append(wm[:, 0:G3])
                wih_sb.append(wm[:, G3:2 * G3])
                xm = xp.tile([128, 2 * c0w], f16, tag=f"x{kp}", name=f"xm{kp}")
                nc.sync.dma_start(
                    xm[:].rearrange("p (k c) -> p k c", k=2),
                    xT.ap()[kp * 128:(kp + 2) * 128, cs0]
                        .rearrange("(k p) b -> p k b", p=128))
                x0_t.append(xm[:, 0:c0w])
                x0_t.append(xm[:, c0w:2 * c0w])
                if kp == 4:
                    bias_sb = cp.tile([128, 5 * ST], f32, tag="bias")
                    nc.sync.dma_start(bias_sb[:], bias.ap())
            brz_sb = bias_sb[:, 0:2 * ST]
            bzn_sb = bias_sb[:, 2 * ST:3 * ST]
            bin_sb = bias_sb[:, 3 * ST:4 * ST]
            bhn_sb = bias_sb[:, 4 * ST:5 * ST]
            whm = cp.tile([128, 2 * G3], f16, tag="whm")
            nc.sync.dma_start(
                whm[:].rearrange("p (k g) -> p k g", k=2),
                whh.ap().rearrange("(k p) g -> p k g", p=128))
            whh_sb = [whm[:, 0:G3], whm[:, G3:2 * G3]]
            h0m = hp.tile([128, 2 * c0w], f16, tag="h0m")
            nc.sync.dma_start(
                h0m[:].rearrange("p (k c) -> p k c", k=2),
                hT.ap()[:, cs0].rearrange("(k p) b -> p k b", p=128))
            h0_t = [h0m[:, 0:c0w], h0m[:, c0w:2 * c0w]]

            cstart = 0
            for c, cw in enumerate(CHUNKS):
                cs = slice(cstart, cstart + cw)
                cstart += cw
                if c == 0:
                    x_t, h_t = x0_t, h0_t
                else:
                    x_t = []
                    for kp in range(0, KX, 2):
                        xm2 = xp.tile([128, 2 * cw], f16, tag=f"x{kp}",
                                      name=f"xc{kp}")
                        nc.sync.dma_start(
                            xm2[:].rearrange("p (k c) -> p k c", k=2),
                            xT.ap()[kp * 128:(kp + 2) * 128, cs]
                                .rearrange("(k p) b -> p k b", p=128))
                        x_t.append(xm2[:, 0:cw])
                        x_t.append(xm2[:, cw:2 * cw])
                    hm2 = hp.tile([128, 2 * cw], f16, tag="h0m", name="hc")
                    nc.sync.dma_start(
                        hm2[:].rearrange("p (k c) -> p k c", k=2),
                        hT.ap()[:, cs].rearrange("(k p) b -> p k b", p=128))
                    h_t = [hm2[:, 0:cw], hm2[:, cw:2 * cw]]

                # PSUM accumulators. r/z gates take input-proj + hidden-proj
                # into the same bank (only their sum is needed downstream).
                p_rz = [pp.tile([128, cw], f32, tag=f"p{gt}", name=f"prz{gt}")
                        for gt in range(2 * ST)]
                p_in = [pp.tile([128, cw], f32, tag=f"p{2 * ST + t_}", name=f"pin{t_}")
                        for t_ in range(ST)]
                p_hn = [pp.tile([128, cw], f32, tag=f"p{3 * ST + t_}", name=f"phn{t_}")
                        for t_ in range(ST)]

                # Input-side first, k-major, so PE consumption tracks the DMA
                # arrival order (wih[k]/x[k] pairs).  The last x k-tile plus
                # all hidden-side matmuls form per-psum "tail groups" ordered
                # so psums complete staggered: r-gates first (sigmoids start
                # draining banks early), i_n last (shortest post-PE chain).
                def gsl(gt):
                    return slice(gt * 128, (gt + 1) * 128)

                for k in range(KX - 1):
                    for gt in range(2 * ST):
                        nc.tensor.matmul(p_rz[gt][:], wih_sb[k][:, gsl(gt)],
                                         x_t[k][:], start=(k == 0), stop=False)
                    for t_ in range(ST):
                        nc.tensor.matmul(p_in[t_][:], wih_sb[k][:, gsl(4 + t_)],
                                         x_t[k][:], start=(k == 0), stop=False)
                kl = KX - 1
                last = (c == len(CHUNKS) - 1)
                o = op.tile([128, ST * cw], f32, tag="o")

                def r_tail(t_):
                    nc.tensor.matmul(p_rz[t_][:], wih_sb[kl][:, gsl(t_)],
                                     x_t[kl][:], start=False, stop=False)
                    for k in range(KH):
                        nc.tensor.matmul(p_rz[t_][:], whh_sb[k][:, gsl(t_)],
                                         h_t[k][:], start=False, stop=(k == KH - 1))

                def hn_tail(t_):
                    for k in range(KH):
                        nc.tensor.matmul(p_hn[t_][:], whh_sb[k][:, gsl(4 + t_)],
                                         h_t[k][:], start=(k == 0), stop=(k == KH - 1))

                def in_tail(t_):
                    nc.tensor.matmul(p_in[t_][:], wih_sb[kl][:, gsl(4 + t_)],
                                     x_t[kl][:], start=False, stop=True)

                def z_tail(t_):
                    gt = ST + t_
                    nc.tensor.matmul(p_rz[gt][:], wih_sb[kl][:, gsl(gt)],
                                     x_t[kl][:], start=False, stop=False)
                    for k in range(KH):
                        nc.tensor.matmul(p_rz[gt][:], whh_sb[k][:, gsl(gt)],
                                         h_t[k][:], start=False, stop=(k == KH - 1))

                def ew_r(t_):
                    r = gp.tile([128, cw], f32, tag=f"r{t_}", name=f"r{t_}")
                    nc.scalar.activation(r[:], p_rz[t_][:], ACT.Sigmoid,
                                         bias=brz_sb[:, t_:t_ + 1])
                    a = gp.tile([128, cw], f32, tag=f"a{t_}", name=f"a{t_}")
                    nc.vector.scalar_tensor_tensor(
                        a[:], p_hn[t_][:], bhn_sb[:, t_:t_ + 1], r[:],
                        ALU.add, ALU.mult)
                    return a

                def ew_z(t_):
                    z = gp.tile([128, cw], f32, tag=f"z{t_}", name=f"z{t_}")
                    nc.scalar.activation(z[:], p_rz[ST + t_][:], ACT.Sigmoid,
                                         bias=brz_sb[:, ST + t_:ST + t_ + 1])
                    zc = gp.tile([128, cw], f32, tag=f"zc{t_}", name=f"zc{t_}")
                    nc.gpsimd.tensor_scalar(zc[:], z[:], -1.0, 1.0,
                                            ALU.mult, ALU.add)
                    return z, zc

                def ew_zh(t_, z):
                    zh = gp.tile([128, cw], f32, tag=f"zh{t_}", name=f"zh{t_}")
                    nc.vector.tensor_mul(zh[:], z[:], h_t[t_][:])
                    return zh

                def ew_tanh(t_, a):
                    b2 = gp.tile([128, cw], f32, tag=f"b{t_}", name=f"b{t_}")
                    nc.vector.tensor_add(b2[:], a[:], p_in[t_][:])
                    n_ = gp.tile([128, cw], f32, tag=f"n{t_}", name=f"n{t_}")
                    nc.scalar.activation(n_[:], b2[:], ACT.Tanh,
                                         bias=bin_sb[:, t_:t_ + 1])
                    return n_

                def ew_out(t_, n_, zc, zh):
                    e = gp.tile([128, cw], f32, tag=f"e{t_}", name=f"e{t_}")
                    nc.vector.tensor_mul(e[:], n_[:], zc[:])
                    nc.vector.tensor_add(o[:, t_ * cw:(t_ + 1) * cw], e[:],
                                         zh[:])

                if not last:
                    # staggered psum completion: r-gates first (sigmoids free
                    # banks for the next chunk), i_n last (short post chain)
                    for t_ in range(ST):
                        r_tail(t_)
                    for t_ in range(ST):
                        hn_tail(t_)
                    for t_ in range(ST):
                        z_tail(t_)
                    for t_ in range(ST):
                        in_tail(t_)
                    as_ = [ew_r(t_) for t_ in range(ST)]
                    zzc = [ew_z(t_) for t_ in range(ST)]
                    zhs = [ew_zh(t_, zzc[t_][0]) for t_ in range(ST)]
                    ns_ = [ew_tanh(t_, as_[t_]) for t_ in range(ST)]
                    for t_ in range(ST):
                        ew_out(t_, ns_[t_], zzc[t_][1], zhs[t_])
                    nc.scalar.dma_start(
                        oT.ap().rearrange("(t p) b -> p t b", p=128)[:, :, cs],
                        o[:].rearrange("p (t c) -> p t c", t=ST))
                else:
                    # final chunk: i_n psums complete before the z-gates so
                    # the b2/tanh chain runs under the last matmuls; b2 goes
                    # ahead of zh on the VectorE queue; per-tile output DMAs
                    # on the scalar and sync DGE queues.
                    for t_ in range(ST):
                        r_tail(t_)
                    for t_ in range(ST):
                        hn_tail(t_)
                    for t_ in range(ST):
                        in_tail(t_)
                    for t_ in range(ST):
                        z_tail(t_)
                    as_ = [ew_r(t_) for t_ in range(ST)]
                    zzc = [ew_z(t_) for t_ in range(ST)]
                    ns_ = [ew_tanh(t_, as_[t_]) for t_ in range(ST)]
                    zhs = [ew_zh(t_, zzc[t_][0]) for t_ in range(ST)]
                    for t_ in range(ST):
                        ew_out(t_, ns_[t_], zzc[t_][1], zhs[t_])
                        eng = nc.scalar if t_ == 0 else nc.sync
                        eng.dma_start(
                            oT.ap()[t_ * 128:(t_ + 1) * 128, cs],
                            o[:, t_ * cw:(t_ + 1) * cw])

    nc.compile()
    return nc


def _get_nc():
    global _cached
    if _cached is None:
        _cached = _build()
    return _cached


def kernel(input, hidden, W_ih, W_hh, b_ih, b_hh):
    input = np.asarray(input, dtype=np.float32)
    hidden = np.asarray(hidden, dtype=np.float32)
    W_ih = np.asarray(W_ih, dtype=np.float32)
    W_hh = np.asarray(W_hh, dtype=np.float32)
    b_ih = np.asarray(b_ih, dtype=np.float32)
    b_hh = np.asarray(b_hh, dtype=np.float32)

    nc = _get_nc()
    from concourse.bass_utils import run_bass_kernel_spmd

    xT = np.ascontiguousarray(input.T.astype(np.float16))
    in_maps = []
    for n in range(NUM_BLOCKS):
        brz_n = (b_ih[n, :2 * BS] + b_hh[n, :2 * BS]).reshape(2 * ST, 128).T
        bzn_n = -brz_n[:, ST:]
        bin_n = b_ih[n, 2 * BS:].reshape(ST, 128).T
        bhn_n = b_hh[n, 2 * BS:].reshape(ST, 128).T
        bias_n = np.concatenate([brz_n, bzn_n, bin_n, bhn_n], axis=1)
        in_maps.append({
            "xT": xT,
            "wih": np.ascontiguousarray(W_ih[n].T.astype(np.float16)),
            "whh": np.ascontiguousarray(W_hh[n].T.astype(np.float16)),
            "hT": np.ascontiguousarray(hidden[:, n * BS:(n + 1) * BS].T.astype(np.float16)),
            "bias": np.ascontiguousarray(bias_n),
        })

    res = run_bass_kernel_spmd(nc, in_maps, core_ids=list(range(NUM_BLOCKS)))
    out = np.empty((BATCH, HIDDEN_DIM), dtype=np.float32)
    for n in range(NUM_BLOCKS):
        out[:, n * BS:(n + 1) * BS] = res.results[n]["oT"].T
    return out



# revision 4
# speedup vs baseline: 1.1453x; 1.1453x over previous
"""BlockGRU Trainium2 kernel — fp8 DoubleRow edition.

Block-diagonal GRU cell: 8 independent blocks (block_size 256), batch 2048,
input_dim 1024. Sharded one block per NeuronCore (8 cores).

All matmuls run in fp8 e4m3 with MatmulPerfMode.DoubleRow (0.5 cycles per
output row, two 128-deep k-tiles per instruction -> 4x the fp16 PE rate in
the cost model). Precision is recovered with residual ("split hi/lo")
correction terms, applied only where the end-to-end error needs them:

  gi = x8@W8 (+ xr8@W8 on z,n gates) (+ x8@WR8 on the n gate)
  gh = h8@Wh8 (+ hr8@Wh8 on the n gate)

where x8 = e4m3(16*x), xr8 = e4m3(16*x - x8), W8 = e4m3(1024*W),
WR8 = e4m3(1024*W - W8), h8 = e4m3(16*h), hr8 = e4m3(16*h - h8). All PSUM
pre-activations share one scale 16384, folded into the ScalarE activation
`scale` operand. The h used by the z*(h-n) output path is reconstructed
on-chip as (h8 + hr8)/16 (rel err ~1e-3). Measured end-to-end rel-L2 error
vs the fp32 reference: ~1.2e-2 (gate: 2e-2). Output DMAs as fp16 and is
upcast on the host.

Per-core layout: gates on partitions, batch on the free dimension. Batch is
processed in chunks sized to PSUM (8 banks). Elementwise work is spread
over ScalarE (sigmoid/tanh), VectorE (r-gating, n pre-add, h reconstruct)
and Pool/GpSimd (output combine) so every engine stays under the PE time.
"""

import sys

if "/opt/trn_rl_repo" not in sys.path:
    sys.path.insert(0, "/opt/trn_rl_repo")

import numpy as np
import ml_dtypes

INPUT_DIM = 1024
HIDDEN_DIM = 2048
NUM_BLOCKS = 8
BS = HIDDEN_DIM // NUM_BLOCKS  # 256
G3 = 3 * BS                    # 768
BATCH = 2048
CHUNKS = [256, 512, 512, 512, 256]
JX = 4                         # input-side k-pairs (8 k-tiles, DoubleRow'd)
ST = 2                         # state partition-tiles per block
SX = 16.0                      # x / h quantization scale
SW = 1024.0                    # weight quantization scale
Q = SX * SW                    # psum pre-activation scale
INV = 1.0 / Q

E4 = ml_dtypes.float8_e4m3

_cached = None


def _build():
    import concourse.tile as tile
    import concourse.mybir as mybir
    from concourse import bacc

    f32 = mybir.dt.float32
    f16 = mybir.dt.float16
    f8 = mybir.dt.float8e4
    ALU = mybir.AluOpType
    ACT = mybir.ActivationFunctionType
    DR = mybir.MatmulPerfMode.DoubleRow

    nc = bacc.Bacc("TRN2", target_bir_lowering=False, debug=False, num_devices=8)

    # DRAM tensors. Free-dim layouts are pre-packed on the host so every DMA
    # lands >=512B-contiguous runs (fp8 would otherwise pay the 2x
    # small-element DMA penalty).
    x8d = nc.dram_tensor("x8", [128, 8 * BATCH], f8, kind="ExternalInput")
    xr8d = nc.dram_tensor("xr8", [128, 8 * BATCH], f8, kind="ExternalInput")
    hpd = nc.dram_tensor("hp", [128, 4 * BATCH], f8, kind="ExternalInput")
    w8d = nc.dram_tensor("w8", [128, JX * 2 * G3], f8, kind="ExternalInput")
    wr8d = nc.dram_tensor("wr8", [128, JX * 2 * BS], f8, kind="ExternalInput")
    wh8d = nc.dram_tensor("wh8", [128, 2 * G3], f8, kind="ExternalInput")
    biasd = nc.dram_tensor("bias", [128, 8], f32, kind="ExternalInput")
    oT = nc.dram_tensor("oT", [BS, BATCH], f16, kind="ExternalOutput")

    with tile.TileContext(nc) as tc:
        with (
            tc.tile_pool(name="const", bufs=1) as cp,
            tc.tile_pool(name="xin", bufs=1) as xp,
            tc.tile_pool(name="xrin", bufs=1) as xrp,
            tc.tile_pool(name="hin", bufs=1) as hp,
            tc.tile_pool(name="gates", bufs=2) as gp,
            tc.tile_pool(name="outs", bufs=1) as op,
            tc.tile_pool(name="psum", bufs=1, space="PSUM") as pp,
        ):
            # PE warm-up: matmuls on a zeroed tile while the prefill DMA
            # runs, so the p-state clock ramp completes before real work.
            wu = cp.tile([128, 32], f16, tag="wu")
            nc.vector.memset(wu[:], 0.0)
            pdummy = pp.tile([128, 32], f32, tag="p0", name="pdummy")
            for _ in range(48):
                nc.tensor.matmul(pdummy[0:32, :], wu[:, 0:32], wu[:],
                                 start=True, stop=True)

            # --- DMA prologue, in PE-consumption order (the input queue is
            # serial at HBM bandwidth). ---
            c0 = CHUNKS[0]
            w_sb = []
            x0_sb = []
            for j in range(JX):
                wj = cp.tile([128, 2 * G3], f8, tag=f"w{j}")
                nc.sync.dma_start(wj[:], w8d.ap()[:, j * 2 * G3:(j + 1) * 2 * G3])
                w_sb.append(wj)
                xj = xp.tile([128, 2 * c0], f8, tag=f"x8c0j{j}")
                nc.sync.dma_start(xj[:], x8d.ap()[:, j * 2 * c0:(j + 1) * 2 * c0])
                x0_sb.append(xj)
            wht = cp.tile([128, 2 * G3], f8, tag="wh")
            nc.sync.dma_start(wht[:], wh8d.ap())
            bt = cp.tile([128, 8], f32, tag="bias")
            nc.sync.dma_start(bt[:], biasd.ap())
            hp0 = hp.tile([128, 4 * c0], f8, tag="hpc0")
            nc.sync.dma_start(hp0[:], hpd.ap()[:, 0:4 * c0])
            xr0 = xrp.tile([128, 8 * c0], f8, tag="xr8c0")
            nc.sync.dma_start(xr0[:], xr8d.ap()[:, 0:8 * c0])
            wrt = cp.tile([128, JX * 2 * BS], f8, tag="wr")
            nc.sync.dma_start(wrt[:], wr8d.ap())
            # Prefetch all remaining chunks' inputs now; the serial queue
            # drains them in order while the PE works.
            xc_sb, xrc_sb, hpc_sb = {}, {}, {}
            cstart = c0
            for c in range(1, len(CHUNKS)):
                cw = CHUNKS[c]
                xc = xp.tile([128, 8 * cw], f8, tag=f"x8c{c}")
                nc.sync.dma_start(xc[:], x8d.ap()[:, 8 * cstart:8 * (cstart + cw)])
                xc_sb[c] = xc
                hpc = hp.tile([128, 4 * cw], f8, tag=f"hpc{c}")
                nc.sync.dma_start(hpc[:], hpd.ap()[:, 4 * cstart:4 * (cstart + cw)])
                hpc_sb[c] = hpc
                xrc = xrp.tile([128, 8 * cw], f8, tag=f"xr8c{c}")
                nc.sync.dma_start(xrc[:], xr8d.ap()[:, 8 * cstart:8 * (cstart + cw)])
                xrc_sb[c] = xrc
                cstart += cw

            def wap(j, gt):      # stationary [128, 2, 128] for gate-tile gt
                return (w_sb[j][:].rearrange("p (k g) -> p k g", k=2)
                        [:, :, gt * 128:(gt + 1) * 128])

            def wrap_(j, t_):    # W-residual stationary, n-gate tile t_
                return (wrt[:, j * 2 * BS:(j + 1) * 2 * BS]
                        .rearrange("p (k g) -> p k g", k=2)
                        [:, :, t_ * 128:(t_ + 1) * 128])

            def whap(gt):        # hidden stationary
                return (wht[:].rearrange("p (k g) -> p k g", k=2)
                        [:, :, gt * 128:(gt + 1) * 128])

            cstart = 0
            for c, cw in enumerate(CHUNKS):
                last = (c == len(CHUNKS) - 1)
                if c == 0:
                    def xap(j, cw=cw):
                        return x0_sb[j][:].rearrange("p (k b) -> p k b", k=2)
                else:
                    def xap(j, cw=cw, c=c):
                        return (xc_sb[c][:, j * 2 * cw:(j + 1) * 2 * cw]
                                .rearrange("p (k b) -> p k b", k=2))
                if c == 0:
                    hpc = hp0
                else:
                    hpc = hpc_sb[c]

                def xrap(j, cw=cw, c=c):
                    t = xr0 if c == 0 else xrc_sb[c]
                    return (t[:, j * 2 * cw:(j + 1) * 2 * cw]
                            .rearrange("p (k b) -> p k b", k=2))

                h8mov = hpc[:, 0:2 * cw].rearrange("p (k b) -> p k b", k=2)
                hr8mov = hpc[:, 2 * cw:4 * cw].rearrange("p (k b) -> p k b", k=2)

                p_r = [pp.tile([128, cw], f32, tag=f"p{t_}", name=f"pr{t_}")
                       for t_ in range(ST)]
                p_z = [pp.tile([128, cw], f32, tag=f"p{ST + t_}", name=f"pz{t_}")
                       for t_ in range(ST)]
                p_in = [pp.tile([128, cw], f32, tag=f"p{2 * ST + t_}", name=f"pin{t_}")
                        for t_ in range(ST)]
                p_hn = [pp.tile([128, cw], f32, tag=f"p{3 * ST + t_}", name=f"phn{t_}")
                        for t_ in range(ST)]

                # T1 (x8 @ W8) k-major; r/z tiles first so the previous
                # chunk's tanh chain can drain p_in before we restart it.
                for j in range(JX):
                    for gt in range(4):
                        psum = p_r[gt] if gt < 2 else p_z[gt - 2]
                        nc.tensor.matmul(psum[:], wap(j, gt), xap(j),
                                         start=(j == 0), stop=False,
                                         perf_mode=DR)
                for j in range(JX):
                    for t_ in range(ST):
                        nc.tensor.matmul(p_in[t_][:], wap(j, 4 + t_), xap(j),
                                         start=(j == 0), stop=False,
                                         perf_mode=DR)
                # r tails: hidden-side h8 projection completes the r psums.
                for t_ in range(ST):
                    nc.tensor.matmul(p_r[t_][:], whap(t_), h8mov,
                                     start=False, stop=True, perf_mode=DR)
                # hn psums: h8 + hr8 (residual-corrected hidden n projection)
                for t_ in range(ST):
                    nc.tensor.matmul(p_hn[t_][:], whap(4 + t_), h8mov,
                                     start=True, stop=False, perf_mode=DR)
                    nc.tensor.matmul(p_hn[t_][:], whap(4 + t_), hr8mov,
                                     start=False, stop=True, perf_mode=DR)

                def z_tail(t_):
                    for j in range(JX):
                        nc.tensor.matmul(p_z[t_][:], wap(j, 2 + t_), xrap(j),
                                         start=False, stop=False, perf_mode=DR)
                    nc.tensor.matmul(p_z[t_][:], whap(2 + t_), h8mov,
                                     start=False, stop=True, perf_mode=DR)

                def in_tail(t_):
                    for j in range(JX):
                        nc.tensor.matmul(p_in[t_][:], wap(j, 4 + t_), xrap(j),
                                         start=False, stop=False, perf_mode=DR)
                    for j in range(JX):
                        nc.tensor.matmul(p_in[t_][:], wrap_(j, t_), xap(j),
                                         start=False, stop=(j == JX - 1),
                                         perf_mode=DR)

                if not last:
                    for t_ in range(ST):
                        z_tail(t_)
                    for t_ in range(ST):
                        in_tail(t_)
                else:
                    # final chunk: i_n completes before z so the tanh chain
                    # runs under the last z matmuls.
                    for t_ in range(ST):
                        in_tail(t_)
                    for t_ in range(ST):
                        z_tail(t_)

                # --- elementwise ---
                o = op.tile([128, ST * cw], f16, tag=f"o{c}")

                def ew_r(t_):
                    r = gp.tile([128, cw], f32, tag=f"r{t_}", name=f"r{t_}")
                    nc.scalar.activation(r[:], p_r[t_][:], ACT.Sigmoid,
                                         bias=bt[:, t_:t_ + 1], scale=INV)
                    return r

                def ew_hs(t_):
                    hs = gp.tile([128, cw], f32, tag=f"hs{t_}", name=f"hs{t_}")
                    nc.gpsimd.tensor_add(hs[:], hpc[:, t_ * cw:(t_ + 1) * cw],
                                         hpc[:, (2 + t_) * cw:(3 + t_) * cw])
                    return hs

                def ew_a(t_, r):
                    a = gp.tile([128, cw], f32, tag=f"a{t_}", name=f"a{t_}")
                    nc.vector.scalar_tensor_tensor(
                        a[:], p_hn[t_][:], bt[:, 6 + t_:7 + t_], r[:],
                        ALU.add, ALU.mult)
                    return a

                def ew_z(t_):
                    z = gp.tile([128, cw], f32, tag=f"z{t_}", name=f"z{t_}")
                    nc.scalar.activation(z[:], p_z[t_][:], ACT.Sigmoid,
                                         bias=bt[:, 2 + t_:3 + t_], scale=INV)
                    return z

                def ew_b2(t_, a):
                    b2 = gp.tile([128, cw], f32, tag=f"b{t_}", name=f"b{t_}")
                    nc.vector.tensor_add(b2[:], a[:], p_in[t_][:])
                    return b2

                def ew_n(t_, b2):
                    n_ = gp.tile([128, cw], f32, tag=f"n{t_}", name=f"n{t_}")
                    nc.scalar.activation(n_[:], b2[:], ACT.Tanh,
                                         bias=bt[:, 4 + t_:5 + t_], scale=INV)
                    return n_

                def ew_d(t_, hs, n_):
                    d = gp.tile([128, cw], f32, tag=f"d{t_}", name=f"d{t_}")
                    nc.vector.scalar_tensor_tensor(
                        d[:], hs[:], 1.0 / SX, n_[:], ALU.mult, ALU.subtract)
                    return d

                def ew_out(t_, z, d, n_):
                    m = gp.tile([128, cw], f32, tag=f"m{t_}", name=f"m{t_}")
                    nc.gpsimd.tensor_mul(m[:], z[:], d[:])
                    nc.gpsimd.tensor_add(o[:, t_ * cw:(t_ + 1) * cw],
                                         n_[:], m[:])

                rs = [ew_r(t_) for t_ in range(ST)]
                hss = [ew_hs(t_) for t_ in range(ST)]
                as_ = [ew_a(t_, rs[t_]) for t_ in range(ST)]
                if not last:
                    zs = [ew_z(t_) for t_ in range(ST)]
                    b2s = [ew_b2(t_, as_[t_]) for t_ in range(ST)]
                    ns = [ew_n(t_, b2s[t_]) for t_ in range(ST)]
                    ds = [ew_d(t_, hss[t_], ns[t_]) for t_ in range(ST)]
                    for t_ in range(ST):
                        ew_out(t_, zs[t_], ds[t_], ns[t_])
                    cs = slice(cstart, cstart + cw)
                    nc.scalar.dma_start(
                        oT.ap().rearrange("(t p) b -> p t b", p=128)[:, :, cs],
                        o[:].rearrange("p (t c) -> p t c", t=ST))
                else:
                    b2s = [ew_b2(t_, as_[t_]) for t_ in range(ST)]
                    ns = [ew_n(t_, b2s[t_]) for t_ in range(ST)]
                    ds = [ew_d(t_, hss[t_], ns[t_]) for t_ in range(ST)]
                    zs = [ew_z(t_) for t_ in range(ST)]
                    for t_ in range(ST):
                        ew_out(t_, zs[t_], ds[t_], ns[t_])
                        eng = nc.scalar if t_ == 0 else nc.sync
                        eng.dma_start(
                            oT.ap()[t_ * 128:(t_ + 1) * 128,
                                    cstart:cstart + cw],
                            o[:, t_ * cw:(t_ + 1) * cw])
                cstart += cw

    nc.compile()
    return nc


def _get_nc():
    global _cached
    if _cached is None:
        _cached = _build()
    return _cached


def _pack_chunks(v, planes):
    """v: [planes..., 128, BATCH] -> [128, planes*BATCH] with each batch
    chunk's block laid out [p, planes, cw] contiguously."""
    blocks = []
    off = 0
    nplanes = int(np.prod(v.shape[:-2]))
    vv = v.reshape(nplanes, 128, BATCH)
    for cw in CHUNKS:
        blk = vv[:, :, off:off + cw].transpose(1, 0, 2).reshape(128, nplanes * cw)
        blocks.append(blk)
        off += cw
    return np.ascontiguousarray(np.concatenate(blocks, axis=1))


def kernel(input, hidden, W_ih, W_hh, b_ih, b_hh):
    input = np.asarray(input, dtype=np.float32)
    hidden = np.asarray(hidden, dtype=np.float32)
    W_ih = np.asarray(W_ih, dtype=np.float32)
    W_hh = np.asarray(W_hh, dtype=np.float32)
    b_ih = np.asarray(b_ih, dtype=np.float32)
    b_hh = np.asarray(b_hh, dtype=np.float32)

    nc = _get_nc()
    from concourse.bass_utils import run_bass_kernel_spmd

    # input-side quantization (shared by all blocks)
    X = input.T * SX                              # [1024, 2048]
    x8 = X.astype(E4)
    xr8 = (X - x8.astype(np.float32)).astype(E4)
    x8p = _pack_chunks(x8.reshape(JX, 2, 128, BATCH), 8)
    xr8p = _pack_chunks(xr8.reshape(JX, 2, 128, BATCH), 8)

    in_maps = []
    for n in range(NUM_BLOCKS):
        Wi = W_ih[n].T * SW                       # [1024, 768]
        W8 = Wi.astype(E4)
        WR8 = (Wi - W8.astype(np.float32))[:, 2 * BS:].astype(E4)  # n gate
        w8p = np.ascontiguousarray(
            W8.reshape(JX, 2, 128, G3).transpose(2, 0, 1, 3).reshape(128, JX * 2 * G3))
        wr8p = np.ascontiguousarray(
            WR8.reshape(JX, 2, 128, BS).transpose(2, 0, 1, 3).reshape(128, JX * 2 * BS))
        Wh = W_hh[n].T * SW                       # [256, 768]
        wh8p = np.ascontiguousarray(
            Wh.astype(E4).reshape(2, 128, G3).transpose(1, 0, 2).reshape(128, 2 * G3))

        Hb = hidden[:, n * BS:(n + 1) * BS].T * SX  # [256, 2048]
        h8 = Hb.astype(E4)
        hr8 = (Hb - h8.astype(np.float32)).astype(E4)
        hpk = _pack_chunks(
            np.stack([h8, hr8]).reshape(2, 2, 128, BATCH), 4)

        brz = b_ih[n, :2 * BS] + b_hh[n, :2 * BS]          # r,z: fused bias
        bin_ = b_ih[n, 2 * BS:]
        bhnQ = b_hh[n, 2 * BS:] * Q
        bias = np.concatenate([
            brz.reshape(4, 128).T.reshape(128, 4),         # br0 br1 bz0 bz1
            bin_.reshape(2, 128).T.reshape(128, 2),        # bin0 bin1
            bhnQ.reshape(2, 128).T.reshape(128, 2),        # bhnQ0 bhnQ1
        ], axis=1).astype(np.float32)

        in_maps.append({
            "x8": x8p,
            "xr8": xr8p,
            "hp": hpk,
            "w8": w8p,
            "wr8": wr8p,
            "wh8": wh8p,
            "bias": np.ascontiguousarray(bias),
        })

    res = run_bass_kernel_spmd(nc, in_maps, core_ids=list(range(NUM_BLOCKS)))
    out = np.empty((BATCH, HIDDEN_DIM), dtype=np.float32)
    for n in range(NUM_BLOCKS):
        out[:, n * BS:(n + 1) * BS] = res.results[n]["oT"].T.astype(np.float32)
    return out


# revision 5
# speedup vs baseline: 1.4947x; 1.3051x over previous
"""BlockGRU Trainium2 kernel — fp8 DoubleRow edition.

Block-diagonal GRU cell: 8 independent blocks (block_size 256), batch 2048,
input_dim 1024. Sharded one block per NeuronCore (8 cores).

All matmuls run in fp8 e4m3 with MatmulPerfMode.DoubleRow (0.5 cycles per
output row, two 128-deep k-tiles per instruction -> 4x the fp16 PE rate in
the cost model). Precision is recovered with residual ("split hi/lo")
correction terms, applied only where the end-to-end error needs them:

  gi = x8@W8 (+ xr8@W8 on z,n gates) (+ x8@WR8 on the n gate)
  gh = h8@Wh8 (+ hr8@Wh8 on the n gate)

where x8 = e4m3(16*x), xr8 = e4m3(16*x - x8), W8 = e4m3(1024*W),
WR8 = e4m3(1024*W - W8), h8 = e4m3(16*h), hr8 = e4m3(16*h - h8). All PSUM
pre-activations share one scale 16384, folded into the ScalarE activation
`scale` operand. Measured end-to-end rel-L2 error vs the fp32 reference:
~1.2e-2 (gate: 2e-2). Output DMAs as fp16 and is upcast on the host.

Engine split keeps each engine under the PE's ~24us:
  ScalarE: r, z, 1-z (= sigmoid of negated pre-activation), tanh
  VectorE: r-gating stt, n pre-add, zc*n, output add
  Pool:    z*h (off the critical path; GPSIMD tensor ops are ~2x DVE cost)
Output DMAs ride the otherwise-idle SP queue so they never head-of-line
block compute issue. PE order per chunk completes psums in the order the
elementwise pipeline consumes them (r -> hn -> z -> i_n) and starts the
n-gate T1 late so the previous chunk's b2 has drained PSUM.
"""

import sys

if "/opt/trn_rl_repo" not in sys.path:
    sys.path.insert(0, "/opt/trn_rl_repo")

import numpy as np
import ml_dtypes

INPUT_DIM = 1024
HIDDEN_DIM = 2048
NUM_BLOCKS = 8
BS = HIDDEN_DIM // NUM_BLOCKS  # 256
G3 = 3 * BS                    # 768
BATCH = 2048
CHUNKS = [256, 512, 512, 512, 256]
JX = 4                         # input-side k-pairs (8 k-tiles, DoubleRow'd)
ST = 2                         # state partition-tiles per block
SX = 16.0                      # x / h quantization scale
SW = 1024.0                    # weight quantization scale
Q = SX * SW                    # psum pre-activation scale
INV = 1.0 / Q

E4 = ml_dtypes.float8_e4m3

_cached = None


def _build():
    import concourse.tile as tile
    import concourse.mybir as mybir
    from concourse import bacc

    f32 = mybir.dt.float32
    f16 = mybir.dt.float16
    f8 = mybir.dt.float8e4
    ALU = mybir.AluOpType
    ACT = mybir.ActivationFunctionType
    DR = mybir.MatmulPerfMode.DoubleRow

    nc = bacc.Bacc("TRN2", target_bir_lowering=False, debug=False, num_devices=8)

    # DRAM tensors. Free-dim layouts are pre-packed on the host so every DMA
    # lands >=512B-contiguous runs (fp8 would otherwise pay the 2x
    # small-element DMA penalty).
    x8d = nc.dram_tensor("x8", [128, 8 * BATCH], f8, kind="ExternalInput")
    xr8d = nc.dram_tensor("xr8", [128, 8 * BATCH], f8, kind="ExternalInput")
    hpd = nc.dram_tensor("hp", [128, 4 * BATCH], f8, kind="ExternalInput")
    h16d = nc.dram_tensor("h16", [128, 2 * BATCH], f16, kind="ExternalInput")
    w8d = nc.dram_tensor("w8", [128, JX * 2 * G3], f8, kind="ExternalInput")
    wr8d = nc.dram_tensor("wr8", [128, JX * 2 * BS], f8, kind="ExternalInput")
    wh8d = nc.dram_tensor("wh8", [128, 2 * G3], f8, kind="ExternalInput")
    biasd = nc.dram_tensor("bias", [128, 10], f32, kind="ExternalInput")
    oT = nc.dram_tensor("oT", [BS, BATCH], f16, kind="ExternalOutput")

    with tile.TileContext(nc) as tc:
        with (
            tc.tile_pool(name="const", bufs=1) as cp,
            tc.tile_pool(name="xin", bufs=1) as xp,
            tc.tile_pool(name="xrin", bufs=1) as xrp,
            tc.tile_pool(name="hin", bufs=1) as hp,
            tc.tile_pool(name="gates", bufs=2) as gp,
            tc.tile_pool(name="outs", bufs=1) as op,
            tc.tile_pool(name="psum", bufs=1, space="PSUM") as pp,
        ):
            # PE warm-up: matmuls on a zeroed tile while the prefill DMA
            # runs, so the p-state clock ramp completes before real work.
            wu = cp.tile([128, 32], f16, tag="wu")
            nc.vector.memset(wu[:], 0.0)
            pdummy = pp.tile([128, 32], f32, tag="p0", name="pdummy")
            for _ in range(48):
                nc.tensor.matmul(pdummy[0:32, :], wu[:, 0:32], wu[:],
                                 start=True, stop=True)

            # --- DMA prologue, in PE-consumption order (the input queue is
            # serial at HBM bandwidth). ---
            c0 = CHUNKS[0]
            w_sb = []
            x0_sb = []
            for j in range(JX):
                wj = cp.tile([128, 2 * G3], f8, tag=f"w{j}")
                nc.sync.dma_start(wj[:], w8d.ap()[:, j * 2 * G3:(j + 1) * 2 * G3])
                w_sb.append(wj)
                xj = xp.tile([128, 2 * c0], f8, tag=f"x8c0j{j}")
                nc.sync.dma_start(xj[:], x8d.ap()[:, j * 2 * c0:(j + 1) * 2 * c0])
                x0_sb.append(xj)
            wht = cp.tile([128, 2 * G3], f8, tag="wh")
            nc.sync.dma_start(wht[:], wh8d.ap())
            bt = cp.tile([128, 10], f32, tag="bias")
            nc.sync.dma_start(bt[:], biasd.ap())
            hp0 = hp.tile([128, 4 * c0], f8, tag="hpc0")
            nc.sync.dma_start(hp0[:], hpd.ap()[:, 0:4 * c0])
            xr0 = xrp.tile([128, 8 * c0], f8, tag="xr8c0")
            nc.sync.dma_start(xr0[:], xr8d.ap()[:, 0:8 * c0])
            wrt = cp.tile([128, JX * 2 * BS], f8, tag="wr")
            nc.sync.dma_start(wrt[:], wr8d.ap())
            h160 = hp.tile([128, 2 * c0], f16, tag="h16c0")
            nc.sync.dma_start(h160[:], h16d.ap()[:, 0:2 * c0])
            # Prefetch all remaining chunks' inputs now; the serial queue
            # drains them in order while the PE works.
            xc_sb, xrc_sb, hpc_sb, h16_sb = {}, {}, {}, {0: h160}
            cstart = c0
            for c in range(1, len(CHUNKS)):
                cw = CHUNKS[c]
                xc = xp.tile([128, 8 * cw], f8, tag=f"x8c{c}")
                nc.sync.dma_start(xc[:], x8d.ap()[:, 8 * cstart:8 * (cstart + cw)])
                xc_sb[c] = xc
                hpc = hp.tile([128, 4 * cw], f8, tag=f"hpc{c}")
                nc.sync.dma_start(hpc[:], hpd.ap()[:, 4 * cstart:4 * (cstart + cw)])
                hpc_sb[c] = hpc
                xrc = xrp.tile([128, 8 * cw], f8, tag=f"xr8c{c}")
                nc.sync.dma_start(xrc[:], xr8d.ap()[:, 8 * cstart:8 * (cstart + cw)])
                xrc_sb[c] = xrc
                h16c = hp.tile([128, 2 * cw], f16, tag=f"h16c{c}")
                nc.sync.dma_start(h16c[:], h16d.ap()[:, 2 * cstart:2 * (cstart + cw)])
                h16_sb[c] = h16c
                cstart += cw

            def wap(j, gt):      # stationary [128, 2, 128] for gate-tile gt
                return (w_sb[j][:].rearrange("p (k g) -> p k g", k=2)
                        [:, :, gt * 128:(gt + 1) * 128])

            def wrap_(j, t_):    # W-residual stationary, n-gate tile t_
                return (wrt[:, j * 2 * BS:(j + 1) * 2 * BS]
                        .rearrange("p (k g) -> p k g", k=2)
                        [:, :, t_ * 128:(t_ + 1) * 128])

            def whap(gt):        # hidden stationary
                return (wht[:].rearrange("p (k g) -> p k g", k=2)
                        [:, :, gt * 128:(gt + 1) * 128])

            cstart = 0
            for c, cw in enumerate(CHUNKS):
                last = (c == len(CHUNKS) - 1)
                if c == 0:
                    def xap(j, cw=cw):
                        return x0_sb[j][:].rearrange("p (k b) -> p k b", k=2)
                else:
                    def xap(j, cw=cw, c=c):
                        return (xc_sb[c][:, j * 2 * cw:(j + 1) * 2 * cw]
                                .rearrange("p (k b) -> p k b", k=2))
                hpc = hp0 if c == 0 else hpc_sb[c]
                h16c = h16_sb[c]

                def xrap(j, cw=cw, c=c):
                    t = xr0 if c == 0 else xrc_sb[c]
                    return (t[:, j * 2 * cw:(j + 1) * 2 * cw]
                            .rearrange("p (k b) -> p k b", k=2))

                h8mov = hpc[:, 0:2 * cw].rearrange("p (k b) -> p k b", k=2)
                hr8mov = hpc[:, 2 * cw:4 * cw].rearrange("p (k b) -> p k b", k=2)

                p_r = [pp.tile([128, cw], f32, tag=f"p{t_}", name=f"pr{t_}")
                       for t_ in range(ST)]
                p_z = [pp.tile([128, cw], f32, tag=f"p{ST + t_}", name=f"pz{t_}")
                       for t_ in range(ST)]
                p_in = [pp.tile([128, cw], f32, tag=f"p{2 * ST + t_}", name=f"pin{t_}")
                        for t_ in range(ST)]
                p_hn = [pp.tile([128, cw], f32, tag=f"p{3 * ST + t_}", name=f"phn{t_}")
                        for t_ in range(ST)]

                # T1 (x8 @ W8) k-major for r/z; the n-gate T1 comes after the
                # z tails so p_in restarts only once the previous chunk's b2
                # has read it, and z completes mid-chunk (its sigmoid frees
                # the bank before the next chunk needs it).
                for j in range(JX):
                    for gt in range(4):
                        psum = p_r[gt] if gt < 2 else p_z[gt - 2]
                        nc.tensor.matmul(psum[:], wap(j, gt), xap(j),
                                         start=(j == 0), stop=False,
                                         perf_mode=DR)
                # r tails: hidden-side h8 projection completes the r psums.
                for t_ in range(ST):
                    nc.tensor.matmul(p_r[t_][:], whap(t_), h8mov,
                                     start=False, stop=True, perf_mode=DR)
                # hn psums: h8 + hr8 (residual-corrected hidden n projection)
                for t_ in range(ST):
                    nc.tensor.matmul(p_hn[t_][:], whap(4 + t_), h8mov,
                                     start=True, stop=False, perf_mode=DR)
                    nc.tensor.matmul(p_hn[t_][:], whap(4 + t_), hr8mov,
                                     start=False, stop=True, perf_mode=DR)
                # z tails: x-residual + hidden projection.
                for t_ in range(ST):
                    for j in range(JX):
                        nc.tensor.matmul(p_z[t_][:], wap(j, 2 + t_), xrap(j),
                                         start=False, stop=False, perf_mode=DR)
                    nc.tensor.matmul(p_z[t_][:], whap(2 + t_), h8mov,
                                     start=False, stop=True, perf_mode=DR)
                # n-gate T1 + tails (x-residual + W-residual), i_n stops last.
                for j in range(JX):
                    for t_ in range(ST):
                        nc.tensor.matmul(p_in[t_][:], wap(j, 4 + t_), xap(j),
                                         start=(j == 0), stop=False,
                                         perf_mode=DR)
                for t_ in range(ST):
                    for j in range(JX):
                        nc.tensor.matmul(p_in[t_][:], wap(j, 4 + t_), xrap(j),
                                         start=False, stop=False, perf_mode=DR)
                    for j in range(JX):
                        nc.tensor.matmul(p_in[t_][:], wrap_(j, t_), xap(j),
                                         start=False, stop=(j == JX - 1),
                                         perf_mode=DR)

                # --- elementwise ---
                # out = zc*n + z*h   with zc = 1-z = sigmoid(-pre_z)
                o = op.tile([128, ST * cw], f16, tag=f"o{c}")

                r_t, zc_t, z_t, zh_t, a_t, b2_t, n_t, e_t = ({} for _ in range(8))

                def ew_r(t_):
                    r = gp.tile([128, cw], f32, tag=f"r{t_}", name=f"r{t_}")
                    nc.scalar.activation(r[:], p_r[t_][:], ACT.Sigmoid,
                                         bias=bt[:, t_:t_ + 1], scale=INV)
                    r_t[t_] = r

                def ew_a(t_):
                    a = gp.tile([128, cw], f32, tag=f"a{t_}", name=f"a{t_}")
                    nc.vector.scalar_tensor_tensor(
                        a[:], p_hn[t_][:], bt[:, 8 + t_:9 + t_], r_t[t_][:],
                        ALU.add, ALU.mult)
                    a_t[t_] = a

                def ew_zc(t_):
                    zc = gp.tile([128, cw], f32, tag=f"zc{t_}", name=f"zc{t_}")
                    nc.scalar.activation(zc[:], p_z[t_][:], ACT.Sigmoid,
                                         bias=bt[:, 4 + t_:5 + t_], scale=-INV)
                    zc_t[t_] = zc

                def ew_z(t_):
                    z = gp.tile([128, cw], f32, tag=f"z{t_}", name=f"z{t_}")
                    nc.scalar.activation(z[:], p_z[t_][:], ACT.Sigmoid,
                                         bias=bt[:, 2 + t_:3 + t_], scale=INV)
                    z_t[t_] = z

                def ew_zh(t_):
                    zh = gp.tile([128, cw], f32, tag=f"zh{t_}", name=f"zh{t_}")
                    nc.gpsimd.tensor_mul(zh[:], z_t[t_][:],
                                         h16c[:, t_ * cw:(t_ + 1) * cw])
                    zh_t[t_] = zh

                def ew_b2(t_):
                    b2 = gp.tile([128, cw], f32, tag=f"b{t_}", name=f"b{t_}")
                    nc.vector.tensor_add(b2[:], a_t[t_][:], p_in[t_][:])
                    b2_t[t_] = b2

                def ew_n(t_):
                    n_ = gp.tile([128, cw], f32, tag=f"n{t_}", name=f"n{t_}")
                    nc.scalar.activation(n_[:], b2_t[t_][:], ACT.Tanh,
                                         bias=bt[:, 6 + t_:7 + t_], scale=INV)
                    n_t[t_] = n_

                def ew_e(t_):
                    e = gp.tile([128, cw], f32, tag=f"e{t_}", name=f"e{t_}")
                    nc.vector.tensor_mul(e[:], zc_t[t_][:], n_t[t_][:])
                    e_t[t_] = e

                def ew_out(t_):
                    nc.vector.tensor_add(o[:, t_ * cw:(t_ + 1) * cw],
                                         e_t[t_][:], zh_t[t_][:])

                cs = slice(cstart, cstart + cw)
                for t_ in range(ST):
                    ew_r(t_)
                for t_ in range(ST):
                    ew_a(t_)
                for t_ in range(ST):
                    ew_zc(t_)
                    ew_z(t_)
                for t_ in range(ST):
                    ew_zh(t_)
                for t_ in range(ST):
                    ew_b2(t_)
                for t_ in range(ST):
                    ew_n(t_)
                if not last:
                    for t_ in range(ST):
                        ew_e(t_)
                        ew_out(t_)
                    nc.sync.dma_start(
                        oT.ap().rearrange("(t p) b -> p t b", p=128)[:, :, cs],
                        o[:].rearrange("p (t c) -> p t c", t=ST))
                else:
                    for t_ in range(ST):
                        ew_e(t_)
                        ew_out(t_)
                        nc.sync.dma_start(
                            oT.ap()[t_ * 128:(t_ + 1) * 128, cs],
                            o[:, t_ * cw:(t_ + 1) * cw])
                cstart += cw

    nc.compile()
    return nc


def _get_nc():
    global _cached
    if _cached is None:
        _cached = _build()
    return _cached


def _pack_chunks(v):
    """v: [planes..., 128, BATCH] -> [128, planes*BATCH] with each batch
    chunk's block laid out [p, planes, cw] contiguously."""
    nplanes = int(np.prod(v.shape[:-2]))
    vv = v.reshape(nplanes, 128, BATCH)
    blocks = []
    off = 0
    for cw in CHUNKS:
        blk = vv[:, :, off:off + cw].transpose(1, 0, 2).reshape(128, nplanes * cw)
        blocks.append(blk)
        off += cw
    return np.ascontiguousarray(np.concatenate(blocks, axis=1))


def kernel(input, hidden, W_ih, W_hh, b_ih, b_hh):
    input = np.asarray(input, dtype=np.float32)
    hidden = np.asarray(hidden, dtype=np.float32)
    W_ih = np.asarray(W_ih, dtype=np.float32)
    W_hh = np.asarray(W_hh, dtype=np.float32)
    b_ih = np.asarray(b_ih, dtype=np.float32)
    b_hh = np.asarray(b_hh, dtype=np.float32)

    nc = _get_nc()
    from concourse.bass_utils import run_bass_kernel_spmd

    # input-side quantization (shared by all blocks)
    X = input.T * SX                              # [1024, 2048]
    x8 = X.astype(E4)
    xr8 = (X - x8.astype(np.float32)).astype(E4)
    x8p = _pack_chunks(x8.reshape(JX, 2, 128, BATCH))
    xr8p = _pack_chunks(xr8.reshape(JX, 2, 128, BATCH))

    in_maps = []
    for n in range(NUM_BLOCKS):
        Wi = W_ih[n].T * SW                       # [1024, 768]
        W8 = Wi.astype(E4)
        WR8 = (Wi - W8.astype(np.float32))[:, 2 * BS:].astype(E4)  # n gate
        w8p = np.ascontiguousarray(
            W8.reshape(JX, 2, 128, G3).transpose(2, 0, 1, 3).reshape(128, JX * 2 * G3))
        wr8p = np.ascontiguousarray(
            WR8.reshape(JX, 2, 128, BS).transpose(2, 0, 1, 3).reshape(128, JX * 2 * BS))
        Wh = W_hh[n].T * SW                       # [256, 768]
        wh8p = np.ascontiguousarray(
            Wh.astype(E4).reshape(2, 128, G3).transpose(1, 0, 2).reshape(128, 2 * G3))

        Hb = hidden[:, n * BS:(n + 1) * BS].T     # [256, 2048]
        Hs = Hb * SX
        h8 = Hs.astype(E4)
        hr8 = (Hs - h8.astype(np.float32)).astype(E4)
        hpk = _pack_chunks(np.stack([h8, hr8]).reshape(2, 2, 128, BATCH))
        h16p = _pack_chunks(
            Hb.astype(np.float16).reshape(2, 128, BATCH))

        brz = b_ih[n, :2 * BS] + b_hh[n, :2 * BS]          # r,z: fused bias
        bin_ = b_ih[n, 2 * BS:]
        bhnQ = b_hh[n, 2 * BS:] * Q
        bz = brz[BS:]
        bias = np.concatenate([
            brz[:BS].reshape(2, 128).T,                    # br0 br1
            bz.reshape(2, 128).T,                          # bz0 bz1
            -bz.reshape(2, 128).T,                         # bnegz0 bnegz1
            bin_.reshape(2, 128).T,                        # bin0 bin1
            bhnQ.reshape(2, 128).T,                        # bhnQ0 bhnQ1
        ], axis=1).astype(np.float32)

        in_maps.append({
            "x8": x8p,
            "xr8": xr8p,
            "hp": hpk,
            "h16": h16p,
            "w8": w8p,
            "wr8": wr8p,
            "wh8": wh8p,
            "bias": np.ascontiguousarray(bias),
        })

    res = run_bass_kernel_spmd(nc, in_maps, core_ids=list(range(NUM_BLOCKS)))
    out = np.empty((BATCH, HIDDEN_DIM), dtype=np.float32)
    for n in range(NUM_BLOCKS):
        out[:, n * BS:(n + 1) * BS] = res.results[n]["oT"].T.astype(np.float32)
    return out


# revision 6
# speedup vs baseline: 1.5581x; 1.0424x over previous
"""BlockGRU Trainium2 kernel — fp8 DoubleRow edition.

Block-diagonal GRU cell: 8 independent blocks (block_size 256), batch 2048,
input_dim 1024. Sharded one block per NeuronCore (8 cores).

All matmuls run in fp8 e4m3 with MatmulPerfMode.DoubleRow (0.5 cycles per
output row, two 128-deep k-tiles per instruction -> 4x the fp16 PE rate in
the cost model). Precision is recovered with residual ("split hi/lo")
correction terms, applied only where the end-to-end error needs them:

  gi = x8@W8 (+ xr8@W8 on z,n gates) (+ x8@WR8 on the n gate)
  gh = h8@Wh8 (+ hr8@Wh8 on the n gate)

where x8 = e4m3(16*x), xr8 = e4m3(16*x - x8), W8 = e4m3(1024*W),
WR8 = e4m3(1024*W - W8), h8 = e4m3(16*h), hr8 = e4m3(16*h - h8). All PSUM
pre-activations share one scale 16384, folded into the ScalarE activation
`scale` operand. The h used by the z*h output path is reconstructed
on-chip as (h8 + hr8)/16 (adds ~1e-3 rel err). Measured end-to-end rel-L2
error vs the fp32 reference: ~1.2e-2 (gate: 2e-2). Output is fp16, upcast
on the host.

Output form: out = zc*n + z*h with zc = 1-z = sigmoid(-pre_z), computed as
a fourth ScalarE activation so the post-tanh critical path is only
e = zc*n -> out = e + zh (zh = z*h is ready before the tanh lands).

Engine split keeps every engine under the PE's ~24us:
  ScalarE: r, zc, z, tanh        VectorE: r-gating stt, zh stt, n pre-add,
  Pool:    h8+hr8, final add              zc*n
Output DMAs ride the otherwise-idle SP queue so they never head-of-line
block compute issue. PE order per chunk completes psums in elementwise
consumption order (r -> hn -> z -> i_n) and starts the n-gate T1 late so
the previous chunk's b2 has drained PSUM. Input DMAs are merged/ordered so
arrival tracks first-use (the serial HWDGE costs ~625ns per DMA).
"""

import sys

if "/opt/trn_rl_repo" not in sys.path:
    sys.path.insert(0, "/opt/trn_rl_repo")

import numpy as np
import ml_dtypes

INPUT_DIM = 1024
HIDDEN_DIM = 2048
NUM_BLOCKS = 8
BS = HIDDEN_DIM // NUM_BLOCKS  # 256
G3 = 3 * BS                    # 768
BATCH = 2048
CHUNKS = [256, 512, 512, 512, 256]
JX = 4                         # input-side k-pairs (8 k-tiles, DoubleRow'd)
ST = 2                         # state partition-tiles per block
SX = 16.0                      # x / h quantization scale
SW = 1024.0                    # weight quantization scale
Q = SX * SW                    # psum pre-activation scale
INV = 1.0 / Q
MISC = 2 * G3 + JX * 2 * BS + 40   # wh | wr | bias bytes per partition

E4 = ml_dtypes.float8_e4m3

_cached = None


def _build():
    import concourse.tile as tile
    import concourse.mybir as mybir
    from concourse import bacc

    f32 = mybir.dt.float32
    f16 = mybir.dt.float16
    f8 = mybir.dt.float8e4
    u8 = mybir.dt.uint8
    ALU = mybir.AluOpType
    ACT = mybir.ActivationFunctionType
    DR = mybir.MatmulPerfMode.DoubleRow

    nc = bacc.Bacc("TRN2", target_bir_lowering=False, debug=False, num_devices=8)

    # DRAM tensors. Free-dim layouts are pre-packed on the host so every DMA
    # lands >=512B-contiguous runs (fp8 would otherwise pay the 2x
    # small-element DMA penalty). sA carries per-chunk [x8 | h8 | hr8]
    # blocks; xr8 streams separately (it is consumed mid-chunk).
    sAd = nc.dram_tensor("sA", [128, 12 * BATCH], f8, kind="ExternalInput")
    xr8d = nc.dram_tensor("xr8", [128, 8 * BATCH], f8, kind="ExternalInput")
    w8d = nc.dram_tensor("w8", [128, JX * 2 * G3], f8, kind="ExternalInput")
    miscd = nc.dram_tensor("misc", [128, MISC], u8, kind="ExternalInput")
    oT = nc.dram_tensor("oT", [BS, BATCH], f16, kind="ExternalOutput")

    with tile.TileContext(nc) as tc:
        with (
            tc.tile_pool(name="const", bufs=1) as cp,
            tc.tile_pool(name="xin", bufs=1) as xp,
            tc.tile_pool(name="xrin", bufs=1) as xrp,
            tc.tile_pool(name="gates", bufs=2) as gp,
            tc.tile_pool(name="outs", bufs=1) as op,
            tc.tile_pool(name="psum", bufs=1, space="PSUM") as pp,
        ):
            # PE warm-up: matmuls on a zeroed tile while the prefill DMA
            # runs, so the p-state clock ramp completes before real work.
            wu = cp.tile([128, 32], f16, tag="wu")
            nc.vector.memset(wu[:], 0.0)
            pdummy = pp.tile([128, 32], f32, tag="p0", name="pdummy")
            for _ in range(48):
                nc.tensor.matmul(pdummy[0:32, :], wu[:, 0:32], wu[:],
                                 start=True, stop=True)

            # --- DMA prologue, ordered by first PE use. ---
            c0 = CHUNKS[0]
            w_sb = []
            # w j0 | x0 j0 | w j1 | x0 j1 | w j23 | x0 j23 -> earliest T1 start
            for j in range(2):
                wj = cp.tile([128, 2 * G3], f8, tag=f"w{j}")
                nc.sync.dma_start(wj[:], w8d.ap()[:, j * 2 * G3:(j + 1) * 2 * G3])
                w_sb.append(wj)
                xj = xp.tile([128, 2 * c0], f8, tag=f"x8c0j{j}")
                nc.sync.dma_start(xj[:], sAd.ap()[:, j * 2 * c0:(j + 1) * 2 * c0])
                if j == 0:
                    x0_a, x0_b = xj, None
                else:
                    x0_b = xj
            w23 = cp.tile([128, 4 * G3], f8, tag="w23")
            nc.sync.dma_start(w23[:], w8d.ap()[:, 4 * G3:8 * G3])
            w_sb += [None, None]
            x023 = xp.tile([128, 4 * c0], f8, tag="x8c0j23")
            nc.sync.dma_start(x023[:], sAd.ap()[:, 4 * c0:8 * c0])
            misc = cp.tile([128, MISC], u8, tag="misc")
            nc.sync.dma_start(misc[:], miscd.ap())
            wht = misc[:, 0:2 * G3].bitcast(f8)
            wrt = misc[:, 2 * G3:2 * G3 + JX * 2 * BS].bitcast(f8)
            bt = misc[:, 2 * G3 + JX * 2 * BS:MISC].bitcast(f32)
            hp0 = xp.tile([128, 4 * c0], f8, tag="hpc0")
            nc.sync.dma_start(hp0[:], sAd.ap()[:, 8 * c0:12 * c0])
            xr0 = xrp.tile([128, 8 * c0], f8, tag="xr8c0")
            nc.sync.dma_start(xr0[:], xr8d.ap()[:, 0:8 * c0])
            # Remaining chunks: one [x8|h8|hr8] DMA plus one xr8 DMA each,
            # interleaved so arrival tracks consumption.
            sA_sb, xrc_sb = {}, {}
            cstart = c0
            for c in range(1, len(CHUNKS)):
                cw = CHUNKS[c]
                sc = xp.tile([128, 12 * cw], f8, tag=f"sAc{c}")
                nc.sync.dma_start(sc[:], sAd.ap()[:, 12 * cstart:12 * (cstart + cw)])
                sA_sb[c] = sc
                xrc = xrp.tile([128, 8 * cw], f8, tag=f"xr8c{c}")
                nc.sync.dma_start(xrc[:], xr8d.ap()[:, 8 * cstart:8 * (cstart + cw)])
                xrc_sb[c] = xrc
                cstart += cw

            def wap(j, gt):      # stationary [128, 2, 128] for gate-tile gt
                if j < 2:
                    t = w_sb[j][:]
                else:
                    t = w23[:, (j - 2) * 2 * G3:(j - 1) * 2 * G3]
                return (t.rearrange("p (k g) -> p k g", k=2)
                        [:, :, gt * 128:(gt + 1) * 128])

            def wrap_(j, t_):    # W-residual stationary, n-gate tile t_
                return (wrt[:, j * 2 * BS:(j + 1) * 2 * BS]
                        .rearrange("p (k g) -> p k g", k=2)
                        [:, :, t_ * 128:(t_ + 1) * 128])

            def whap(gt):        # hidden stationary
                return (wht.rearrange("p (k g) -> p k g", k=2)
                        [:, :, gt * 128:(gt + 1) * 128])

            cstart = 0
            for c, cw in enumerate(CHUNKS):
                last = (c == len(CHUNKS) - 1)
                if c == 0:
                    def xap(j, cw=cw):
                        if j == 0:
                            t = x0_a[:]
                        elif j == 1:
                            t = x0_b[:]
                        else:
                            t = x023[:, (j - 2) * 2 * cw:(j - 1) * 2 * cw]
                        return t.rearrange("p (k b) -> p k b", k=2)
                    hpc = hp0[:]
                else:
                    def xap(j, cw=cw, c=c):
                        return (sA_sb[c][:, j * 2 * cw:(j + 1) * 2 * cw]
                                .rearrange("p (k b) -> p k b", k=2))
                    hpc = sA_sb[c][:, 8 * cw:12 * cw]

                def xrap(j, cw=cw, c=c):
                    t = xr0 if c == 0 else xrc_sb[c]
                    return (t[:, j * 2 * cw:(j + 1) * 2 * cw]
                            .rearrange("p (k b) -> p k b", k=2))

                h8mov = hpc[:, 0:2 * cw].rearrange("p (k b) -> p k b", k=2)
                hr8mov = hpc[:, 2 * cw:4 * cw].rearrange("p (k b) -> p k b", k=2)

                p_r = [pp.tile([128, cw], f32, tag=f"p{t_}", name=f"pr{t_}")
                       for t_ in range(ST)]
                p_z = [pp.tile([128, cw], f32, tag=f"p{ST + t_}", name=f"pz{t_}")
                       for t_ in range(ST)]
                p_in = [pp.tile([128, cw], f32, tag=f"p{2 * ST + t_}", name=f"pin{t_}")
                        for t_ in range(ST)]
                p_hn = [pp.tile([128, cw], f32, tag=f"p{3 * ST + t_}", name=f"phn{t_}")
                        for t_ in range(ST)]

                # T1 (x8 @ W8) k-major for r/z; the n-gate T1 comes after the
                # z tails so p_in restarts only once the previous chunk's b2
                # has read it, and z completes mid-chunk (its sigmoid frees
                # the bank before the next chunk needs it).
                for j in range(JX):
                    for gt in range(4):
                        psum = p_r[gt] if gt < 2 else p_z[gt - 2]
                        nc.tensor.matmul(psum[:], wap(j, gt), xap(j),
                                         start=(j == 0), stop=False,
                                         perf_mode=DR)
                # r tails: hidden-side h8 projection completes the r psums.
                for t_ in range(ST):
                    nc.tensor.matmul(p_r[t_][:], whap(t_), h8mov,
                                     start=False, stop=True, perf_mode=DR)
                # hn psums: h8 + hr8 (residual-corrected hidden n projection)
                for t_ in range(ST):
                    nc.tensor.matmul(p_hn[t_][:], whap(4 + t_), h8mov,
                                     start=True, stop=False, perf_mode=DR)
                    nc.tensor.matmul(p_hn[t_][:], whap(4 + t_), hr8mov,
                                     start=False, stop=True, perf_mode=DR)
                # z tails: x-residual + hidden projection.
                for t_ in range(ST):
                    for j in range(JX):
                        nc.tensor.matmul(p_z[t_][:], wap(j, 2 + t_), xrap(j),
                                         start=False, stop=False, perf_mode=DR)
                    nc.tensor.matmul(p_z[t_][:], whap(2 + t_), h8mov,
                                     start=False, stop=True, perf_mode=DR)
                # n-gate T1 + tails (x-residual + W-residual), i_n stops last.
                for j in range(JX):
                    for t_ in range(ST):
                        nc.tensor.matmul(p_in[t_][:], wap(j, 4 + t_), xap(j),
                                         start=(j == 0), stop=False,
                                         perf_mode=DR)
                for t_ in range(ST):
                    for j in range(JX):
                        nc.tensor.matmul(p_in[t_][:], wap(j, 4 + t_), xrap(j),
                                         start=False, stop=False, perf_mode=DR)
                    for j in range(JX):
                        nc.tensor.matmul(p_in[t_][:], wrap_(j, t_), xap(j),
                                         start=False, stop=(j == JX - 1),
                                         perf_mode=DR)

                # --- elementwise:  out = zc*n + z*h,  zc = sigmoid(-pre) ---
                o = op.tile([128, ST * cw], f16, tag=f"o{c}")
                r_t, zc_t, z_t, zh_t, a_t, b2_t, n_t, e_t, hs_t = \
                    ({} for _ in range(9))

                def ew_r(t_):
                    r = gp.tile([128, cw], f32, tag=f"r{t_}", name=f"r{t_}")
                    nc.scalar.activation(r[:], p_r[t_][:], ACT.Sigmoid,
                                         bias=bt[:, t_:t_ + 1], scale=INV)
                    r_t[t_] = r

                def ew_hs(t_):
                    hs = gp.tile([128, cw], f32, tag=f"hs{t_}", name=f"hs{t_}")
                    nc.gpsimd.tensor_add(hs[:], hpc[:, t_ * cw:(t_ + 1) * cw],
                                         hpc[:, (2 + t_) * cw:(3 + t_) * cw])
                    hs_t[t_] = hs

                def ew_a(t_):
                    a = gp.tile([128, cw], f32, tag=f"a{t_}", name=f"a{t_}")
                    nc.vector.scalar_tensor_tensor(
                        a[:], p_hn[t_][:], bt[:, 8 + t_:9 + t_], r_t[t_][:],
                        ALU.add, ALU.mult)
                    a_t[t_] = a

                def ew_zc_act(t_):
                    zc = gp.tile([128, cw], f32, tag=f"zc{t_}", name=f"zc{t_}")
                    nc.scalar.activation(zc[:], p_z[t_][:], ACT.Sigmoid,
                                         bias=bt[:, 4 + t_:5 + t_], scale=-INV)
                    zc_t[t_] = zc

                def ew_zc_pool(t_):
                    zc = gp.tile([128, cw], f32, tag=f"zc{t_}", name=f"zc{t_}")
                    nc.gpsimd.tensor_scalar(zc[:], z_t[t_][:], -1.0, 1.0,
                                            ALU.mult, ALU.add)
                    zc_t[t_] = zc

                def ew_z(t_):
                    z = gp.tile([128, cw], f32, tag=f"z{t_}", name=f"z{t_}")
                    nc.scalar.activation(z[:], p_z[t_][:], ACT.Sigmoid,
                                         bias=bt[:, 2 + t_:3 + t_], scale=INV)
                    z_t[t_] = z

                def ew_zh(t_):
                    zh = gp.tile([128, cw], f32, tag=f"zh{t_}", name=f"zh{t_}")
                    nc.vector.scalar_tensor_tensor(
                        zh[:], hs_t[t_][:], 1.0 / SX, z_t[t_][:],
                        ALU.mult, ALU.mult)
                    zh_t[t_] = zh

                def ew_b2(t_):
                    b2 = gp.tile([128, cw], f32, tag=f"b{t_}", name=f"b{t_}")
                    nc.vector.tensor_add(b2[:], a_t[t_][:], p_in[t_][:])
                    b2_t[t_] = b2

                def ew_n(t_):
                    n_ = gp.tile([128, cw], f32, tag=f"n{t_}", name=f"n{t_}")
                    nc.scalar.activation(n_[:], b2_t[t_][:], ACT.Tanh,
                                         bias=bt[:, 6 + t_:7 + t_], scale=INV)
                    n_t[t_] = n_

                def ew_e(t_):
                    e = gp.tile([128, cw], f32, tag=f"e{t_}", name=f"e{t_}")
                    nc.vector.tensor_mul(e[:], zc_t[t_][:], n_t[t_][:])
                    e_t[t_] = e

                def ew_out(t_):
                    nc.gpsimd.tensor_add(o[:, t_ * cw:(t_ + 1) * cw],
                                         e_t[t_][:], zh_t[t_][:])

                cs = slice(cstart, cstart + cw)
                for t_ in range(ST):
                    ew_r(t_)
                for t_ in range(ST):
                    ew_hs(t_)
                for t_ in range(ST):
                    ew_a(t_)
                if not last:
                    for t_ in range(ST):
                        ew_zc_act(t_)
                        ew_z(t_)
                else:
                    for t_ in range(ST):
                        ew_z(t_)
                    for t_ in range(ST):
                        ew_zc_pool(t_)
                for t_ in range(ST):
                    ew_zh(t_)
                for t_ in range(ST):
                    ew_b2(t_)
                for t_ in range(ST):
                    ew_n(t_)
                for t_ in range(ST):
                    ew_e(t_)
                    ew_out(t_)
                    if last:
                        nc.sync.dma_start(
                            oT.ap()[t_ * 128:(t_ + 1) * 128, cs],
                            o[:, t_ * cw:(t_ + 1) * cw])
                if not last:
                    nc.sync.dma_start(
                        oT.ap().rearrange("(t p) b -> p t b", p=128)[:, :, cs],
                        o[:].rearrange("p (t c) -> p t c", t=ST))
                cstart += cw

    nc.compile()
    return nc


def _get_nc():
    global _cached
    if _cached is None:
        _cached = _build()
    return _cached


def _pack_chunks(v):
    """v: [planes, 128, BATCH] -> [128, planes*BATCH] with each batch
    chunk's block laid out [p, planes, cw] contiguously."""
    nplanes = int(np.prod(v.shape[:-2]))
    vv = v.reshape(nplanes, 128, BATCH)
    blocks = []
    off = 0
    for cw in CHUNKS:
        blk = vv[:, :, off:off + cw].transpose(1, 0, 2).reshape(128, nplanes * cw)
        blocks.append(blk)
        off += cw
    return np.concatenate(blocks, axis=1)


def kernel(input, hidden, W_ih, W_hh, b_ih, b_hh):
    input = np.asarray(input, dtype=np.float32)
    hidden = np.asarray(hidden, dtype=np.float32)
    W_ih = np.asarray(W_ih, dtype=np.float32)
    W_hh = np.asarray(W_hh, dtype=np.float32)
    b_ih = np.asarray(b_ih, dtype=np.float32)
    b_hh = np.asarray(b_hh, dtype=np.float32)

    nc = _get_nc()
    from concourse.bass_utils import run_bass_kernel_spmd

    # input-side quantization (shared by all blocks)
    X = input.T * SX                              # [1024, 2048]
    x8 = X.astype(E4)
    xr8 = (X - x8.astype(np.float32)).astype(E4)
    x8v = x8.reshape(8, 128, BATCH)               # [ktile, p, b]
    xr8p = np.ascontiguousarray(_pack_chunks(xr8.reshape(8, 128, BATCH)))

    in_maps = []
    for n in range(NUM_BLOCKS):
        Wi = W_ih[n].T * SW                       # [1024, 768]
        W8 = Wi.astype(E4)
        WR8 = (Wi - W8.astype(np.float32))[:, 2 * BS:].astype(E4)  # n gate
        w8p = np.ascontiguousarray(
            W8.reshape(JX, 2, 128, G3).transpose(2, 0, 1, 3).reshape(128, JX * 2 * G3))
        wr8p = WR8.reshape(JX, 2, 128, BS).transpose(2, 0, 1, 3).reshape(128, JX * 2 * BS)
        Wh = W_hh[n].T * SW                       # [256, 768]
        wh8p = Wh.astype(E4).reshape(2, 128, G3).transpose(1, 0, 2).reshape(128, 2 * G3)

        Hs = hidden[:, n * BS:(n + 1) * BS].T * SX  # [256, 2048]
        h8 = Hs.astype(E4)
        hr8 = (Hs - h8.astype(np.float32)).astype(E4)
        # sA: per chunk [x8(8 planes) | h8(2) | hr8(2)]
        sA = _pack_chunks(np.concatenate(
            [x8v, h8.reshape(2, 128, BATCH), hr8.reshape(2, 128, BATCH)]))

        brz = b_ih[n, :2 * BS] + b_hh[n, :2 * BS]          # r,z: fused bias
        bz = brz[BS:]
        bias = np.concatenate([
            brz[:BS].reshape(2, 128).T,                    # br0 br1
            bz.reshape(2, 128).T,                          # bz0 bz1
            -bz.reshape(2, 128).T,                         # bnegz0 bnegz1
            b_ih[n, 2 * BS:].reshape(2, 128).T,            # bin0 bin1
            (b_hh[n, 2 * BS:] * Q).reshape(2, 128).T,      # bhnQ0 bhnQ1
        ], axis=1).astype(np.float32)
        misc = np.concatenate([
            wh8p.view(np.uint8), wr8p.view(np.uint8),
            np.ascontiguousarray(bias).view(np.uint8).reshape(128, 40),
        ], axis=1)

        in_maps.append({
            "sA": np.ascontiguousarray(sA),
            "xr8": xr8p,
            "w8": w8p,
            "misc": np.ascontiguousarray(misc),
        })

    res = run_bass_kernel_spmd(nc, in_maps, core_ids=list(range(NUM_BLOCKS)))
    out = np.empty((BATCH, HIDDEN_DIM), dtype=np.float32)
    for n in range(NUM_BLOCKS):
        out[:, n * BS:(n + 1) * BS] = res.results[n]["oT"].T.astype(np.float32)
    return out
